# revision 1
# baseline (speedup 1.0000x reference)
"""DiagonalBandAttention Trainium2 kernel.

Computation (reference semantics):
  band[b,c,j]  = mean_{k=0..20} xpad[b,c,j+k,j]        (rows zero-padded by 10)
  conv[b,c,s]  = depthwise_conv1d(band, conv_w, k=7, pad=3)   (cross-correlation)
  attn[b,d,s]  = softmax_s( sum_c point_w[d,c]*conv[b,c,s] + point_b[d] )
  out          = x, with out[b,c,j,j] = x[b,c,j,j] * attn[b,c,j]

Output is x copied verbatim except the main diagonal of each [S,S] map.
The kernel is memory-bound on the x -> out copy (2 * 384 MB).

Sharding (8 cores): core k handles batch b = k//4, channels [48*(k%4), 48*(k%4)+48).
Each core:
  - bulk-copies its x shard DRAM->DRAM,
  - receives the diagonal-band slices E[b] = xpad[b,:,j+k,j] of its whole batch
    (all 192 channels are needed because the 1x1 conv mixes channels),
  - computes band-mean -> depthwise conv -> pointwise matmul -> softmax on chip,
  - scatters the rescaled diagonal into the copied output.
"""

import numpy as np

B, C, S = 2, 192, 512
BW = 21          # band width
HALF = BW // 2   # 10
K = 7            # depthwise conv taps
CSH = C // 4     # 48 channels per core
N_CORES = 8
BULK_CH = 4      # channels per bulk copy DMA

_prog = {}


def _build_program(debug=False):
    """Raw-bass program (Tile's sem assignment emits multi-wait compute
    instructions that this walrus rejects, so sync is managed manually).

    Engine plan:
      SP     - 12 big DRAM->DRAM copies x_sh -> out        (bulk sem)
      ACT    - input DMAs, exp, final diagonal scatter      (din/asem)
      DVE    - band sum, depthwise conv, softmax arithmetic (vs)
      PE     - 1x1 conv matmuls into PSUM                   (psem)

    Cross-engine deps (all single-sem standalone waits):
      DVE waits din>=128 (all 8 input DMAs)   -> band/conv -> vs=1
      PE  waits vs>=1                          -> matmuls  -> psem=1
      DVE waits psem>=1                        -> bias+negmax -> vs=3
      ACT waits vs>=3                          -> exp+sum  -> asem=1
      DVE waits asem>=1                        -> dv       -> vs=4
      ACT waits vs>=4 and bulk>=192            -> diag scatter -> din=144
    """
    import concourse.bass as bass
    import concourse.mybir as mybir

    f32 = mybir.dt.float32
    Alu = mybir.AluOpType
    N_BULK = CSH // BULK_CH

    nc = bass.Bass()
    x_sh = nc.declare_dram_parameter("x_sh", [CSH, S, S], f32, isOutput=False)
    e_b = nc.declare_dram_parameter("e_b", [C, BW, S], f32, isOutput=False)
    xdg = nc.declare_dram_parameter("xdg", [CSH, S], f32, isOutput=False)
    cw = nc.declare_dram_parameter("cw", [C, K], f32, isOutput=False)
    pwt = nc.declare_dram_parameter("pwt", [256, CSH], f32, isOutput=False)
    pb = nc.declare_dram_parameter("pb", [CSH, 1], f32, isOutput=False)
    out = nc.declare_dram_parameter("out", [CSH, S, S], f32, isOutput=True)
    dbg = {}
    if debug:
        for name, shape in (
            ("band_o", [128, S + K - 1]), ("ct_o", [128, S]), ("sm_o", [CSH, S]),
            ("ex_o", [CSH, S]), ("ssum_o", [CSH, 1]), ("rinv_o", [CSH, 1]),
            ("dv_o", [CSH, S]),
        ):
            dbg[name] = nc.declare_dram_parameter(name, shape, f32, isOutput=True)

    x_flat = x_sh.ap().rearrange("c h w -> c (h w)")
    out_flat = out.ap().rearrange("c h w -> c (h w)")
    e_ap = e_b.ap()
    cw_ap = cw.ap()
    pwt_ap = pwt.ap()

    from contextlib import ExitStack

    with ExitStack() as ctx:
        et1 = ctx.enter_context(nc.sbuf_tensor([128, BW, S], f32))
        et2 = ctx.enter_context(nc.sbuf_tensor([64, BW, S], f32))
        band1 = ctx.enter_context(nc.sbuf_tensor([128, S + K - 1], f32))
        band2 = ctx.enter_context(nc.sbuf_tensor([64, S + K - 1], f32))
        ct1 = ctx.enter_context(nc.sbuf_tensor([128, S], f32))
        ct2 = ctx.enter_context(nc.sbuf_tensor([128, S], f32))
        cw1 = ctx.enter_context(nc.sbuf_tensor([128, K], f32))
        cw2 = ctx.enter_context(nc.sbuf_tensor([64, K], f32))
        pw1 = ctx.enter_context(nc.sbuf_tensor([128, CSH], f32))
        pw2 = ctx.enter_context(nc.sbuf_tensor([128, CSH], f32))
        pbt = ctx.enter_context(nc.sbuf_tensor([CSH, 1], f32))
        sm = ctx.enter_context(nc.sbuf_tensor([CSH, S], f32))
        negmax = ctx.enter_context(nc.sbuf_tensor([CSH, 1], f32))
        ex = ctx.enter_context(nc.sbuf_tensor([CSH, S], f32))
        ssum = ctx.enter_context(nc.sbuf_tensor([CSH, 1], f32))
        rinv = ctx.enter_context(nc.sbuf_tensor([CSH, 1], f32))
        lse = ctx.enter_context(nc.sbuf_tensor([CSH, 1], f32))
        nrt = ctx.enter_context(nc.sbuf_tensor([CSH, 1], f32))
        xdgt = ctx.enter_context(nc.sbuf_tensor([CSH, S], f32))
        dv = ctx.enter_context(nc.sbuf_tensor([CSH, S], f32))
        ps = ctx.enter_context(nc.psum_tensor([CSH, S], f32))
        din = ctx.enter_context(nc.semaphore("din"))
        bulk = ctx.enter_context(nc.semaphore("bulk"))
        vs = ctx.enter_context(nc.semaphore("vs"))
        psem = ctx.enter_context(nc.semaphore("psem"))
        asem = ctx.enter_context(nc.semaphore("asem"))
        block = ctx.enter_context(nc.Block())

        @block.sync
        def _(sync):
            # inputs first: their completion starves behind bulk packets in
            # the SDMA round-robin otherwise, stalling compute ~400us
            sync.wait_ge(din, 128)
            for i in range(N_BULK):
                sync.dma_start(
                    out=out_flat[i * BULK_CH : (i + 1) * BULK_CH, :],
                    in_=x_flat[i * BULK_CH : (i + 1) * BULK_CH, :],
                ).then_inc(bulk, 16)

        @block.scalar
        def _(scalar):
            scalar.dma_start(out=et1[:], in_=e_ap[0:128]).then_inc(din, 16)
            scalar.dma_start(out=et2[:], in_=e_ap[128:C]).then_inc(din, 16)
            scalar.dma_start(out=cw1[:], in_=cw_ap[0:128]).then_inc(din, 16)
            scalar.dma_start(out=cw2[:], in_=cw_ap[128:C]).then_inc(din, 16)
            scalar.dma_start(out=pw1[:], in_=pwt_ap[0:128]).then_inc(din, 16)
            scalar.dma_start(out=pw2[:], in_=pwt_ap[128:256]).then_inc(din, 16)
            scalar.dma_start(out=pbt[:], in_=pb.ap()).then_inc(din, 16)
            scalar.dma_start(out=xdgt[:], in_=xdg.ap()).then_inc(din, 16)
            scalar.wait_ge(vs, 3)
            scalar.activation(
                out=ex[:], in_=sm[:], func=mybir.ActivationFunctionType.Exp,
                bias=negmax[:], scale=1.0,
            ).then_inc(asem, 1)
            # seed 1/ssum = exp(-ln(ssum)); DVE Newton-polishes it
            scalar.wait_ge(vs, 4)
            scalar.activation(
                out=lse[:], in_=ssum[:], func=mybir.ActivationFunctionType.Ln
            )
            scalar.activation(
                out=rinv[:], in_=lse[:], func=mybir.ActivationFunctionType.Exp,
                scale=-1.0,
            ).then_inc(asem, 1)
            scalar.wait_ge(vs, 5)
            # diagonal scatter per bulk chunk, each ordered after its
            # chunk's copy so the (slow, 4B-RMW) descriptors overlap the
            # remaining bulk instead of serializing at the end
            n_dma = 8 + N_BULK
            with nc.allow_non_contiguous_dma(reason="diagonal scatter"):
                for i in range(N_BULK):
                    scalar.wait_ge(bulk, 16 * (i + 1))
                    scalar.dma_start(
                        out=out_flat[
                            i * BULK_CH : (i + 1) * BULK_CH, 0 : S * S : S + 1
                        ],
                        in_=dv[i * BULK_CH : (i + 1) * BULK_CH, :],
                    ).then_inc(din, 16)
            if debug:
                for name, src in (
                    ("band_o", band1), ("ct_o", ct1), ("sm_o", sm), ("ex_o", ex),
                    ("ssum_o", ssum), ("rinv_o", rinv), ("dv_o", dv),
                ):
                    scalar.dma_start(out=dbg[name].ap(), in_=src[:]).then_inc(din, 16)
                    n_dma += 1
            scalar.wait_ge(din, 16 * n_dma)

        @block.vector
        def _(vector):
            vector.wait_ge(din, 128)
            # band sums over the 21 taps (mean's 1/21 folded into cw on host)
            for (band, et, p) in ((band1, et1, 128), (band2, et2, 64)):
                bs = band[0:p, 3 : 3 + S]
                vector.tensor_tensor(
                    out=bs, in0=et[0:p, 0, :], in1=et[0:p, 1, :], op=Alu.add
                )
                for k in range(2, BW):
                    vector.tensor_tensor(
                        out=bs, in0=et[0:p, k, :], in1=bs, op=Alu.add
                    )
                vector.memset(band[0:p, 0:3], 0.0)
                vector.memset(band[0:p, 3 + S :], 0.0)
            vector.memset(ct2[64:128, :], 0.0)  # zero padding partitions
            # depthwise conv, 7 taps
            for (ct, band, cwt, p) in ((ct1, band1, cw1, 128), (ct2, band2, cw2, 64)):
                vector.tensor_scalar(
                    out=ct[0:p, :], in0=band[0:p, 0:S],
                    scalar1=cwt[0:p, 0:1], scalar2=None, op0=Alu.mult,
                )
                for t in range(1, K):
                    stt = vector.scalar_tensor_tensor(
                        out=ct[0:p, :], in0=band[0:p, t : t + S],
                        scalar=cwt[0:p, t : t + 1], in1=ct[0:p, :],
                        op0=Alu.mult, op1=Alu.add,
                    )
                stt.then_inc(vs, 1)  # vs=1 after ct1, vs=2 after ct2
            vector.wait_ge(psem, 1)
            vector.tensor_scalar_add(out=sm[:], in0=ps[:], scalar1=pbt[:])
            vector.tensor_reduce(
                out=negmax[:], in_=sm[:], axis=mybir.AxisListType.X,
                op=Alu.max, negate=True,
            ).then_inc(vs, 1)  # vs=3: exp inputs ready
            vector.wait_ge(asem, 1)
            vector.tensor_reduce(
                out=ssum[:], in_=ex[:], axis=mybir.AxisListType.X, op=Alu.add
            ).then_inc(vs, 1)  # vs=4: ssum ready for ACT's 1/x seed
            vector.wait_ge(asem, 2)
            for _ in range(2):  # Newton: y <- y*(2 - x*y)
                vector.tensor_tensor(
                    out=nrt[:], in0=ssum[:], in1=rinv[:], op=Alu.mult
                )
                vector.tensor_scalar(
                    out=nrt[:], in0=nrt[:], scalar1=-1.0, scalar2=2.0,
                    op0=Alu.mult, op1=Alu.add,
                )
                vector.tensor_tensor(
                    out=rinv[:], in0=rinv[:], in1=nrt[:], op=Alu.mult
                )
            vector.tensor_tensor(out=dv[:], in0=ex[:], in1=xdgt[:], op=Alu.mult)
            vector.tensor_scalar_mul(
                out=dv[:], in0=dv[:], scalar1=rinv[:]
            ).then_inc(vs, 1)  # vs=5: dv ready

        @block.tensor
        def _(tensor):
            tensor.wait_ge(vs, 2)
            nc.tensor.matmul(ps[:], lhsT=pw1[:], rhs=ct1[:], start=True, stop=False)
            nc.tensor.matmul(
                ps[:], lhsT=pw2[:], rhs=ct2[:], start=False, stop=True
            ).then_inc(psem, 1)

    return nc


def _get_program(debug=False):
    if debug not in _prog:
        _prog[debug] = _build_program(debug)
    return _prog[debug]


def _host_prep(x, conv_w, point_w, point_b):
    """Build per-core input maps. Everything here is slicing/layout only."""
    x = np.asarray(x, dtype=np.float32)
    conv_w = np.asarray(conv_w, dtype=np.float32)
    point_w = np.asarray(point_w, dtype=np.float32)
    point_b = np.asarray(point_b, dtype=np.float32)

    # E[b,c,k,j] = xpad[b,c,j+k,j]  (rows padded by HALF), via diagonal views
    E = np.zeros((B, C, BW, S), dtype=np.float32)
    for k in range(BW):
        o = HALF - k
        d = np.diagonal(x, offset=o, axis1=2, axis2=3)
        if o >= 0:
            E[:, :, k, o:S] = d
        else:
            E[:, :, k, 0 : S + o] = d

    cw_all = np.ascontiguousarray(conv_w.reshape(C, K) / np.float32(BW))

    in_maps = []
    for core in range(N_CORES):
        b, cb = divmod(core, 4)
        c0 = cb * CSH
        pwt_sh = np.zeros((256, CSH), dtype=np.float32)
        pwt_sh[:C] = point_w[c0 : c0 + CSH, :].T
        in_maps.append(
            {
                "x_sh": np.ascontiguousarray(x[b, c0 : c0 + CSH]),
                "e_b": np.ascontiguousarray(E[b]),
                "xdg": np.ascontiguousarray(E[b, c0 : c0 + CSH, HALF, :]),
                "cw": cw_all,
                "pwt": pwt_sh,
                "pb": np.ascontiguousarray(point_b[c0 : c0 + CSH].reshape(CSH, 1)),
            }
        )
    return in_maps


def _run(inputs, trace=False, debug=False):
    from concourse.bass_utils import run_bass_kernel_spmd

    nc = _get_program(debug)
    in_maps = _host_prep(**inputs)
    res = run_bass_kernel_spmd(
        nc, in_maps, core_ids=list(range(N_CORES)), trace=trace
    )
    out = np.empty((B, C, S, S), dtype=np.float32)
    for core in range(N_CORES):
        b, cb = divmod(core, 4)
        c0 = cb * CSH
        out[b, c0 : c0 + CSH] = res.results[core]["out"]
    return out, res


def kernel(x, conv_w, point_w, point_b):
    out, _ = _run(dict(x=x, conv_w=conv_w, point_w=point_w, point_b=point_b))
    return out



# revision 4
# speedup vs baseline: 1.3383x; 1.3383x over previous
"""DiagonalBandAttention Trainium2 kernel.

Computation (reference semantics):
  band[b,c,j]  = mean_{k=0..20} xpad[b,c,j+k,j]        (rows zero-padded by 10)
  conv[b,c,s]  = depthwise_conv1d(band, conv_w, k=7, pad=3)   (cross-correlation)
  attn[b,d,s]  = softmax_s( sum_c point_w[d,c]*conv[b,c,s] + point_b[d] )
  out          = x, with out[b,c,j,j] = x[b,c,j,j] * attn[b,c,j]

Output is x copied verbatim except the main diagonal of each [S,S] map.
The kernel is memory-bound on the x -> out copy (2 * 384 MB).

Sharding (8 cores): core k handles batch b = k//4, channels [48*(k%4), 48*(k%4)+48).

v2 design: the x -> out copy is routed THROUGH SBUF (not DRAM->DRAM) so the
diagonal scale is applied on-chip, eliminating the per-element (4B RMW)
diagonal scatter that previously serialized 24576 descriptors onto 4 of the
16 SDMA engines (that scatter accounted for ~280us of the 470us runtime).

Each channel's [512, 512] map is one tile [128 partitions x 2048 f32]:
partition p holds map rows {p, 128+p, 256+p, 384+p} (sub-block i = rows
128i..128i+127, columns 0..511). The diagonal of sub-block i is at tile
columns 640i + p, i.e. an identity-aligned [128,128] block at column 640i.
After softmax, attn is PE-transposed so qm1[p, i*48+c] = attn[c,128i+p]-1,
and the diagonal fix per channel is 3 DVE tensor_tensor ops:
  F[p,i,q]   = qm1[p, i*48+c] * I128[p,q]         (bcast strides)
  F         *= tile[p, 640i+q]                     (strided 4x128 block view)
  tile[...] += F
Stores go back out with the inverse access pattern.

Pipelining: 18 tile slots (8 dedicated + 10 reclaimed from the e_b band
buffers once the band sums consume them). Per-slot load/store semaphores
make the gating exact (cumulative counts on a shared sem are racy because
later DMAs on the same ring can pre-increment it).
"""

import numpy as np

B, C, S = 2, 192, 512
BW = 21          # band width
HALF = BW // 2   # 10
K = 7            # depthwise conv taps
CSH = C // 4     # 48 channels per core
N_CORES = 8

NS = 18          # tile slots: 8 dedicated + 5 in et1 + 5 in et2
NDED = 8
EBF = BW * S     # 10752 f32 per partition of e_b flat

_prog = {}


def _build_program():
    """Raw-bass program (manual semaphores; Tile's multi-wait emission is
    rejected by this walrus).

    Engine plan:
      SP   - 48 tile loads x_re[c] -> slot (per-slot lsem)
      ACT  - 9 input DMAs, softmax exp/ln, 48 tile stores (per-slot ssem)
      DVE  - band sum, depthwise conv, softmax arith, per-channel diag fix
      PE   - 1x1 conv matmuls, 4 attn transposes

    vs milestones (DVE): 1=band sums done (et region free), 2=ct1, 3=ct2,
    4=sm+negmax, 5=ssum, 6=attn_sm ready.
    psem (PE): 1=pointwise matmul, 2=transposes done.
    asem (ACT): 1=exp done, 2=rinv seed done.
    fsem (DVE): +1 per channel diagonal fix.
    din: all 9 input DMAs (wait the full 144 only - partial counts race).
    """
    import concourse.bass as bass
    import concourse.mybir as mybir
    from concourse.ap import AP

    f32 = mybir.dt.float32
    Alu = mybir.AluOpType

    nc = bass.Bass()
    x_sh = nc.declare_dram_parameter("x_sh", [CSH, S, S], f32, isOutput=False)
    e_b = nc.declare_dram_parameter("e_b", [C, EBF], f32, isOutput=False)
    cw = nc.declare_dram_parameter("cw", [C, K], f32, isOutput=False)
    pwt = nc.declare_dram_parameter("pwt", [256, CSH], f32, isOutput=False)
    pb = nc.declare_dram_parameter("pb", [CSH, 1], f32, isOutput=False)
    i48 = nc.declare_dram_parameter("i48", [CSH, CSH], f32, isOutput=False)
    i128 = nc.declare_dram_parameter("i128", [128, 128], f32, isOutput=False)
    out = nc.declare_dram_parameter("out", [CSH, S, S], f32, isOutput=True)

    # partition p <- map rows {128i + p}, free dim = (i, w)
    x_re = x_sh.ap().rearrange("c (i p) w -> c p i w", i=4, p=128)
    out_re = out.ap().rearrange("c (i p) w -> c p i w", i=4, p=128)
    e_ap = e_b.ap()
    cw_ap = cw.ap()
    pwt_ap = pwt.ap()

    from contextlib import ExitStack

    with ExitStack() as ctx:
        ded = ctx.enter_context(nc.sbuf_tensor([128, NDED * 2048], f32))
        et1 = ctx.enter_context(nc.sbuf_tensor([128, EBF], f32))
        et2 = ctx.enter_context(nc.sbuf_tensor([128, EBF], f32))
        band1 = ctx.enter_context(nc.sbuf_tensor([128, S + K - 1], f32))
        band2 = ctx.enter_context(nc.sbuf_tensor([64, S + K - 1], f32))
        ct1 = ctx.enter_context(nc.sbuf_tensor([128, S], f32))
        ct2 = ctx.enter_context(nc.sbuf_tensor([128, S], f32))
        cw1 = ctx.enter_context(nc.sbuf_tensor([128, K], f32))
        cw2 = ctx.enter_context(nc.sbuf_tensor([64, K], f32))
        pw1 = ctx.enter_context(nc.sbuf_tensor([128, CSH], f32))
        pw2 = ctx.enter_context(nc.sbuf_tensor([128, CSH], f32))
        pbt = ctx.enter_context(nc.sbuf_tensor([CSH, 1], f32))
        sm = ctx.enter_context(nc.sbuf_tensor([CSH, S], f32))
        negmax = ctx.enter_context(nc.sbuf_tensor([CSH, 1], f32))
        ex = ctx.enter_context(nc.sbuf_tensor([CSH, S], f32))
        ssum = ctx.enter_context(nc.sbuf_tensor([CSH, 1], f32))
        rinv = ctx.enter_context(nc.sbuf_tensor([CSH, 1], f32))
        lse = ctx.enter_context(nc.sbuf_tensor([CSH, 1], f32))
        nrt = ctx.enter_context(nc.sbuf_tensor([CSH, 1], f32))
        attn = ctx.enter_context(nc.sbuf_tensor([CSH, S], f32))
        i48s = ctx.enter_context(nc.sbuf_tensor([CSH, CSH], f32))
        i128s = ctx.enter_context(nc.sbuf_tensor([128, 128], f32))
        qm1 = ctx.enter_context(nc.sbuf_tensor([128, 4 * CSH], f32))
        fbuf = ctx.enter_context(nc.sbuf_tensor([128, 512], f32))
        ps = ctx.enter_context(nc.psum_tensor([CSH, S], f32))
        psq = ctx.enter_context(nc.psum_tensor([128, 4 * CSH], f32))
        din = ctx.enter_context(nc.semaphore("din"))
        vs = ctx.enter_context(nc.semaphore("vs"))
        psem = ctx.enter_context(nc.semaphore("psem"))
        asem = ctx.enter_context(nc.semaphore("asem"))
        fsem = ctx.enter_context(nc.semaphore("fsem"))
        lsem = [ctx.enter_context(nc.semaphore(f"ls{i}")) for i in range(NS)]
        ssem = [ctx.enter_context(nc.semaphore(f"ss{i}")) for i in range(NS)]
        block = ctx.enter_context(nc.Block())

        def slot_ap(s):
            if s < NDED:
                return ded.ap()[:, s * 2048 : (s + 1) * 2048]
            if s < NDED + 5:
                j = s - NDED
                return et1.ap()[:, j * 2048 : (j + 1) * 2048]
            j = s - NDED - 5
            return et2.ap()[:, j * 2048 : (j + 1) * 2048]

        def diag_ap(s):
            t = slot_ap(s)
            return AP(
                tensor=t.tensor,
                offset=t.offset,
                ap=[list(t.ap[0]), [640, 4], [1, 128]],
            )

        @block.sync
        def _(sync):
            for c in range(CSH):
                s = c % NS
                if c == NDED:
                    sync.wait_ge(vs, 1)  # et1/et2 consumed by band sums
                if c >= NS:
                    sync.wait_ge(ssem[s], 16 * (c // NS))
                sync.dma_start(out=slot_ap(s), in_=x_re[c]).then_inc(lsem[s], 16)

        @block.scalar
        def _(scalar):
            scalar.dma_start(out=et1[:], in_=e_ap[0:128]).then_inc(din, 16)
            scalar.dma_start(out=et2[0:64, :], in_=e_ap[128:C]).then_inc(din, 16)
            scalar.dma_start(out=cw1[:], in_=cw_ap[0:128]).then_inc(din, 16)
            scalar.dma_start(out=cw2[:], in_=cw_ap[128:C]).then_inc(din, 16)
            scalar.dma_start(out=pw1[:], in_=pwt_ap[0:128]).then_inc(din, 16)
            scalar.dma_start(out=pw2[:], in_=pwt_ap[128:256]).then_inc(din, 16)
            scalar.dma_start(out=pbt[:], in_=pb.ap()).then_inc(din, 16)
            scalar.dma_start(out=i48s[:], in_=i48.ap()).then_inc(din, 16)
            scalar.dma_start(out=i128s[:], in_=i128.ap()).then_inc(din, 16)
            scalar.wait_ge(vs, 4)
            scalar.activation(
                out=ex[:], in_=sm[:], func=mybir.ActivationFunctionType.Exp,
                bias=negmax[:], scale=1.0,
            ).then_inc(asem, 1)
            # seed 1/ssum = exp(-ln(ssum)); DVE Newton-polishes it
            scalar.wait_ge(vs, 5)
            scalar.activation(
                out=lse[:], in_=ssum[:], func=mybir.ActivationFunctionType.Ln
            )
            scalar.activation(
                out=rinv[:], in_=lse[:], func=mybir.ActivationFunctionType.Exp,
                scale=-1.0,
            ).then_inc(asem, 1)
            for c in range(CSH):
                s = c % NS
                scalar.wait_ge(fsem, c + 1)
                scalar.dma_start(out=out_re[c], in_=slot_ap(s)).then_inc(
                    ssem[s], 16
                )
            # drain: the kernel must not end with store DMAs in flight
            for s in range(NS):
                n_stores = len(range(s, CSH, NS))
                scalar.wait_ge(ssem[s], 16 * n_stores)

        @block.vector
        def _(vector):
            vector.wait_ge(din, 144)  # all 9 input DMAs (full count: exact)
            # band sums over the 21 taps (mean's 1/21 folded into cw on host)
            for (band, et, p) in ((band1, et1, 128), (band2, et2, 64)):
                bs = band[0:p, 3 : 3 + S]
                vector.tensor_tensor(
                    out=bs, in0=et[0:p, 0:S], in1=et[0:p, S : 2 * S], op=Alu.add
                )
                for k in range(2, BW):
                    vector.tensor_tensor(
                        out=bs, in0=et[0:p, k * S : (k + 1) * S], in1=bs,
                        op=Alu.add,
                    )
            vector.memset(band1[:, 0:3], 0.0)
            vector.memset(band1[:, 3 + S :], 0.0)
            vector.memset(band2[:, 0:3], 0.0)
            vector.memset(band2[:, 3 + S :], 0.0)
            vector.memset(ct2[64:128, :], 0.0).then_inc(vs, 1)  # et region free
            # depthwise conv, 7 taps
            for (ct, band, cwt, p) in ((ct1, band1, cw1, 128), (ct2, band2, cw2, 64)):
                vector.tensor_scalar(
                    out=ct[0:p, :], in0=band[0:p, 0:S],
                    scalar1=cwt[0:p, 0:1], scalar2=None, op0=Alu.mult,
                )
                for t in range(1, K):
                    stt = vector.scalar_tensor_tensor(
                        out=ct[0:p, :], in0=band[0:p, t : t + S],
                        scalar=cwt[0:p, t : t + 1], in1=ct[0:p, :],
                        op0=Alu.mult, op1=Alu.add,
                    )
                stt.then_inc(vs, 1)  # vs=2 after ct1, vs=3 after ct2
            vector.wait_ge(psem, 1)
            vector.tensor_scalar_add(out=sm[:], in0=ps[:], scalar1=pbt[:])
            vector.tensor_reduce(
                out=negmax[:], in_=sm[:], axis=mybir.AxisListType.X,
                op=Alu.max, negate=True,
            ).then_inc(vs, 1)  # vs=4: exp inputs ready
            vector.wait_ge(asem, 1)
            vector.tensor_reduce(
                out=ssum[:], in_=ex[:], axis=mybir.AxisListType.X, op=Alu.add
            ).then_inc(vs, 1)  # vs=5: ssum ready for ACT's 1/x seed
            vector.wait_ge(asem, 2)
            for _ in range(2):  # Newton: y <- y*(2 - x*y)
                vector.tensor_tensor(
                    out=nrt[:], in0=ssum[:], in1=rinv[:], op=Alu.mult
                )
                vector.tensor_scalar(
                    out=nrt[:], in0=nrt[:], scalar1=-1.0, scalar2=2.0,
                    op0=Alu.mult, op1=Alu.add,
                )
                vector.tensor_tensor(
                    out=rinv[:], in0=rinv[:], in1=nrt[:], op=Alu.mult
                )
            vector.tensor_scalar_mul(
                out=attn[:], in0=ex[:], scalar1=rinv[:]
            ).then_inc(vs, 1)  # vs=6: attn ready for PE transposes
            vector.wait_ge(psem, 2)
            vector.tensor_scalar_add(out=qm1[:], in0=psq[:], scalar1=-1.0)
            fb4 = AP(
                tensor=fbuf.ap().tensor, offset=fbuf.ap().offset,
                ap=[list(fbuf.ap().ap[0]), [128, 4], [1, 128]],
            )
            i128b = i128s.ap().unsqueeze(1).to_broadcast([128, 4, 128])
            for c in range(CSH):
                s = c % NS
                vector.wait_ge(lsem[s], 16 * (c // NS + 1))
                dap = diag_ap(s)
                qsb = (
                    qm1.ap()[:, c : 4 * CSH : CSH]
                    .unsqueeze(2)
                    .to_broadcast([128, 4, 128])
                )
                vector.tensor_tensor(out=fb4, in0=qsb, in1=i128b, op=Alu.mult)
                vector.tensor_tensor(out=fb4, in0=fb4, in1=dap, op=Alu.mult)
                vector.tensor_tensor(
                    out=dap, in0=dap, in1=fb4, op=Alu.add
                ).then_inc(fsem, 1)

        @block.tensor
        def _(tensor):
            tensor.wait_ge(din, 144)
            tensor.wait_ge(vs, 3)
            nc.tensor.matmul(ps[:], lhsT=pw1[:], rhs=ct1[:], start=True, stop=False)
            nc.tensor.matmul(
                ps[:], lhsT=pw2[:], rhs=ct2[:], start=False, stop=True
            ).then_inc(psem, 1)
            tensor.wait_ge(vs, 6)
            for i in range(4):
                mm = nc.tensor.matmul(
                    psq[:, i * CSH : (i + 1) * CSH],
                    lhsT=attn[:, i * 128 : (i + 1) * 128],
                    rhs=i48s[:],
                    start=True, stop=True,
                )
            mm.then_inc(psem, 1)  # psem=2: all transposes done

    return nc


def _get_program():
    if "p" not in _prog:
        _prog["p"] = _build_program()
    return _prog["p"]


def _host_prep(x, conv_w, point_w, point_b):
    """Build per-core input maps. Everything here is slicing/layout only."""
    x = np.asarray(x, dtype=np.float32)
    conv_w = np.asarray(conv_w, dtype=np.float32)
    point_w = np.asarray(point_w, dtype=np.float32)
    point_b = np.asarray(point_b, dtype=np.float32)

    # E[b,c,k,j] = xpad[b,c,j+k,j]  (rows padded by HALF), via diagonal views
    E = np.zeros((B, C, BW, S), dtype=np.float32)
    for k in range(BW):
        o = HALF - k
        d = np.diagonal(x, offset=o, axis1=2, axis2=3)
        if o >= 0:
            E[:, :, k, o:S] = d
        else:
            E[:, :, k, 0 : S + o] = d
    E = E.reshape(B, C, EBF)

    cw_all = np.ascontiguousarray(conv_w.reshape(C, K) / np.float32(BW))
    eye48 = np.eye(CSH, dtype=np.float32)
    eye128 = np.eye(128, dtype=np.float32)

    in_maps = []
    for core in range(N_CORES):
        b, cb = divmod(core, 4)
        c0 = cb * CSH
        pwt_sh = np.zeros((256, CSH), dtype=np.float32)
        pwt_sh[:C] = point_w[c0 : c0 + CSH, :].T
        in_maps.append(
            {
                "x_sh": np.ascontiguousarray(x[b, c0 : c0 + CSH]),
                "e_b": np.ascontiguousarray(E[b]),
                "cw": cw_all,
                "pwt": pwt_sh,
                "pb": np.ascontiguousarray(point_b[c0 : c0 + CSH].reshape(CSH, 1)),
                "i48": eye48,
                "i128": eye128,
            }
        )
    return in_maps


def _run(inputs, trace=False):
    from concourse.bass_utils import run_bass_kernel_spmd

    nc = _get_program()
    in_maps = _host_prep(**inputs)
    res = run_bass_kernel_spmd(
        nc, in_maps, core_ids=list(range(N_CORES)), trace=trace
    )
    out = np.empty((B, C, S, S), dtype=np.float32)
    for core in range(N_CORES):
        b, cb = divmod(core, 4)
        c0 = cb * CSH
        out[b, c0 : c0 + CSH] = res.results[core]["out"]
    return out, res


def kernel(x, conv_w, point_w, point_b):
    out, _ = _run(dict(x=x, conv_w=conv_w, point_w=point_w, point_b=point_b))
    return out
